# revision 12
# baseline (speedup 1.0000x reference)
"""Trainium2 Bass kernel for nn_BitGatConv_48524540510805.

Reference computation:
    nhs    = nodes_ft @ weight                      # [n, HC]
    f1     = nhs @ conv_weight1
    f2     = nhs @ conv_weight2
    logits = f1[:, None, :] + f2[None, :, :]        # [n, n, HC]
    scores = leaky_relu(logits) + adj_bias[:, :, None]
    coefs  = softmax(scores, axis=1)                # over source-node dim j
    vals   = sum_j coefs[i, j, c] * nhs[i, c]       # NOTE: nhs[i], not nhs[j]

Because the softmax normalizes over j and the weighted value is nhs[i, c]
(independent of j), the weighted sum telescopes:

    vals[i, c] = nhs[i, c] * sum_j coefs[i, j, c] = nhs[i, c]

(softmax rows always sum to 1; the mask bias is finite so no NaNs, and the
adjacency always includes self-loops anyway).  Verified numerically against
the full reference: max per-element relative error ~8e-7 (pure fp32
rounding of the softmax row-sum).  So the kernel only needs
nhs = nodes_ft @ weight.

Distribution: destination-node rows are sharded across the 8 cores
(192 rows each, per the sharding hint).  Each core computes
outT_c = weight.T @ x_c.T on the tensor engine.

Per-core device program (latency-optimized; every engine/DMA-queue choice
below was driven by TimelineSim cost-model iteration, then validated on
hardware with repeated stress runs):
  - host packs xw_c = [weight | x_c.T] as one [256, 256] f32 array so each
    128-partition k-chunk (row-plane) of the packed tensor holds exactly
    the operands of one accumulating matmul;
  - plane A (k=0:128) loads via the SP HWDGE queue; plane B (k=128:256)
    is split so the k1 matmuls can start as early as possible:
    [w1 | x1 cols 0:64] (512 B/partition) via the gpsimd SWDGE queue in
    parallel with A, and x1 cols 64:192 as a second SP HWDGE DMA
    pipelined behind A;
  - PE accumulates into two PSUM tensors sized to a full 2KB bank each
    (so they land in different banks), split at output column 64: the k0
    matmuls run when plane A lands, the k1 matmuls as their B pieces
    land (start/stop accumulation flags per PSUM range);
  - DVE copies each finished PSUM bank to SBUF as soon as its last
    matmul retires (bank separation avoids the PE-write/DVE-read
    same-bank hazard), signalling v_sem per piece; SP issues the output
    DMA after both.  The copy->DMA semaphore is required for
    correctness: a DMA issued right after a compute op on the same
    engine races it (the sequencer runs ahead of the engine), which
    showed up as intermittent corruption on hardware;
  - per-DMA semaphores throughout (completion order between DMAs from
    one engine is not guaranteed across HW queues);
  - SP waits for the output DMA's completion semaphore before the
    program ends.  This is required: without it, back-to-back executions
    of the NEFF in one process hit NRT_EXEC_UNIT_UNRECOVERABLE (the next
    execution races the prior one's in-flight output DMA at the DGE
    rings);
  - the framework preamble's four const-tensor memsets and the initial
    all-engine barrier are stripped post-build: nothing in this program
    depends on them (constants are never read; all cross-engine
    dependencies are semaphore-gated), and they delay the first DMA
    config by ~500ns.
"""

import numpy as np

import concourse.bass as bass
import concourse.mybir as mybir
from concourse.bass_utils import run_bass_kernel_spmd

N = 1536
IN_CH = 256
HC = 64
N_CORES = 8
ROWS = N // N_CORES  # 192 destination rows per core
PK = HC + ROWS  # 256 packed columns: [w | x_c.T]
H = 64  # output-column split: piece A = cols 0:64, piece B = cols 64:192
XB = HC + H  # packed-column boundary of the SWDGE piece of plane B

_FP32 = mybir.dt.float32

_built = None


def _strip_preamble(nc):
    """Drop framework preamble instructions this program never depends on:
    the four const-tensor memsets (float32-0.0/1.0, bfloat16-1.0,
    uint8-127 — reader-less here, as the BIR verifier notes) and the
    initial all-engine drain+barrier that only existed to order those
    memsets before use.  All inter-engine dependencies in this kernel are
    explicitly semaphore-gated, so engines may enter their blocks
    unsynchronized.  Saves ~500ns of startup latency."""
    blk = nc.m.functions[0].blocks[0]
    keep = []
    for i in blk.instructions:
        tn = type(i).__name__
        if tn == "InstMemset" and i.outs and str(
            getattr(i.outs[0], "memref", "")
        ).startswith("const-"):
            continue
        if tn in ("InstDrain", "InstEventSemaphore"):
            continue
        keep.append(i)
    try:
        blk.instructions[:] = keep
    except TypeError:
        blk.instructions = keep


def _build_bass():
    """Per-core program: outT[HC, ROWS] = w.T @ x_c.T, K=256 split in two."""
    nc = bass.Bass()
    xw = nc.dram_tensor("xw", [IN_CH, PK], _FP32, kind="ExternalInput")
    outT = nc.dram_tensor("outT", [HC, ROWS], _FP32, kind="ExternalOutput")

    with (
        nc.sbuf_tensor("tA", [128, PK], _FP32) as tA,
        nc.sbuf_tensor("tB", [128, PK], _FP32) as tB,
        # 512 f32 = 2KB/partition: each PSUM tensor fills one bank exactly,
        # guaranteeing the two accumulators sit in different banks.
        nc.psum_tensor("ps_a", [HC, 512], _FP32) as ps_a,
        nc.psum_tensor("ps_b", [HC, 512], _FP32) as ps_b,
        nc.sbuf_tensor("o", [HC, ROWS], _FP32) as o,
        nc.semaphore("dma_sem") as dma_sem,
        nc.semaphore("dmb_sem") as dmb_sem,
        nc.semaphore("dmc_sem") as dmc_sem,
        nc.semaphore("pe_sem") as pe_sem,
        nc.semaphore("v_sem") as v_sem,
        nc.Block() as block,
    ):

        @block.sync
        def _(sync):
            sync.dma_start(out=tA[:, :], in_=xw[0:128, :]).then_inc(dma_sem, 16)
            sync.dma_start(out=tB[:, XB:PK], in_=xw[128:256, XB:PK]).then_inc(dmc_sem, 16)
            sync.wait_ge(v_sem, 2)
            sync.dma_start(out=outT[:, :], in_=o[:, :]).then_inc(dma_sem, 16)
            sync.wait_ge(dma_sem, 32)

        @block.gpsimd
        def _(gpsimd):
            gpsimd.dma_start(out=tB[:, 0:XB], in_=xw[128:256, 0:XB]).then_inc(dmb_sem, 16)
            gpsimd.wait_ge(dmb_sem, 16)

        @block.tensor
        def _(tensor):
            tensor.wait_ge(dma_sem, 16)
            tensor.matmul(ps_a[:, 0:H], tA[:, 0:HC], tA[:, HC:XB], start=True, stop=False)
            tensor.matmul(ps_b[:, 0:ROWS - H], tA[:, 0:HC], tA[:, XB:PK], start=True, stop=False)
            tensor.wait_ge(dmb_sem, 16)
            tensor.matmul(
                ps_a[:, 0:H], tB[:, 0:HC], tB[:, HC:XB],
                start=False, stop=True, skip_group_check=True,
            ).then_inc(pe_sem, 1)
            tensor.wait_ge(dmc_sem, 16)
            tensor.matmul(
                ps_b[:, 0:ROWS - H], tB[:, 0:HC], tB[:, XB:PK],
                start=False, stop=True, skip_group_check=True,
            ).then_inc(pe_sem, 1)

        @block.vector
        def _(vector):
            vector.wait_ge(pe_sem, 1)
            vector.tensor_copy(o[:, 0:H], ps_a[:, 0:H]).then_inc(v_sem, 1)
            vector.wait_ge(pe_sem, 2)
            vector.tensor_copy(o[:, H:ROWS], ps_b[:, 0:ROWS - H]).then_inc(v_sem, 1)

    _strip_preamble(nc)
    return nc


def _shard_inputs(nodes_ft, weight):
    nodes_ft = np.ascontiguousarray(nodes_ft, dtype=np.float32)
    w = np.ascontiguousarray(weight, dtype=np.float32)
    in_maps = []
    for c in range(N_CORES):
        x_c = nodes_ft[c * ROWS : (c + 1) * ROWS, :]  # [ROWS, IN_CH]
        xw = np.empty((IN_CH, PK), dtype=np.float32)
        xw[:, 0:HC] = w
        xw[:, HC:PK] = x_c.T
        in_maps.append({"xw": xw})
    return in_maps


def kernel(nodes_ft, adj_bias_mat, weight, conv_weight1, conv_weight2):
    global _built
    if _built is None:
        _built = _build_bass()

    in_maps = _shard_inputs(nodes_ft, weight)
    res = run_bass_kernel_spmd(_built, in_maps, list(range(N_CORES)))

    out = np.empty((N, HC), dtype=np.float32)
    for c in range(N_CORES):
        out[c * ROWS : (c + 1) * ROWS, :] = res.results[c]["outT"].T
    return out
